# revision 40
# baseline (speedup 1.0000x reference)
"""GroupQueryAttention (softmax over the GROUP axis) on 8 trn2 NeuronCores.

Reference computation (B=2, S=2048, D=1024, G=8, h=128):
    q = hidden @ Wq + bq ; k = hidden @ Wk + bk ; v = hidden @ Wv + bv
    scores[b,n,m,g] = sum_h q[b,n,g,h] k[b,m,g,h] / sqrt(D)
    probs = softmax(scores, axis=g)            # couples groups per (n,m)
    ctx[b,n,g,h] = sum_m probs[b,n,m,g] v[b,m,g,h]

Sharding: 2 batches x 4 query-blocks of 512 = 8 cores. Each core
recomputes its batch's full K,V (collectives measured ~50us per 4MB
AllGather on this fleet - slower than recompute).

v3 (from v1=262us, v2=359us):
  - K projection in fp8e4m3 DoubleRow (2x PE): host sends X and 32*Wk
    in fp8; PSUM scale 1/32 recovers k. Q and V stay 16-bit: host-sim
    shows fp8 on Q+K = 1.5e-2 and any fp8 on V >= 1.5e-2 max-err,
    vs 6.5e-3 for K-only (budget 2e-2).
  - fp16 instead of bf16 for all 16-bit tensors (same speed, 4x less
    quantization error) - buys the margin that K-fp8 spends.
  - V weights pre-scaled by 32 (fp16), bias 32*bv added via a
    prebuilt broadcast tile on DVE during PSUM evacuation; the 1/32
    folds into the ctx PSUM evacuation scale. No PE rank-1 bias mms.
  - Engine balance from measured rates (ACT .83ns/el, DVE .52ns/el
    16-bit SBUF, Pool ~3ns/el, PSUM unreachable from Pool):
    exp + Q/K copies + ctx evac on ACT; Z-tree + recip + V-bias-adds
    + most muls on DVE; 1/Z cast + pass-1 mul tail on Pool.
  - Softmax chain per m-tile PAIR; ctx matmuls of the previous pair
    and one V-projection unit interleave behind each scores half
    (V unit first so PE never waits on the softmax chain).
  - k_chunk(0) + Q emitted first with a minimal DMA prefix on the
    sync queue; wq/wv/xtb arrive in parallel on gpsimd/scalar queues.

Per-core layouts (SBUF partition dim first):
    xt8  (128,8,2048) fp8   X^T, d=dt*128+p, cols rolled queries-first
    xtb  (128,8,2048) fp16  same, for Q and V projections
    Q^T  (128,8,512)  fp16  [h, g, n] scaled 1/32 (+bq/32)
    K^T  (128,8,2048) fp16  [h, g, m] (+bk)
    V    (128,16,1024) fp16 [m, mt, g*128+h] = 32*(XWv+bv)
    e2   (128,2,8,256) fp16 exp tiles for an m-tile pair
Output: ctxT (1024, 512) fp16 per core; host transposes/upcasts.
"""

import os

os.environ.setdefault("JAX_COMPILATION_CACHE_DIR", "/tmp/jax_comp_cache")

import numpy as np
import ml_dtypes

import concourse.bass as bass
import concourse.mybir as mybir
import concourse.tile as tile
from concourse import bacc
from concourse.bass_utils import run_bass_kernel_spmd

F16 = mybir.dt.float16
FP8 = mybir.dt.float8e4
F32 = mybir.dt.float32
DR = mybir.MatmulPerfMode.DoubleRow

B, S, D, G = 2, 2048, 1024, 8
H = D // G          # 128, group head dim
NQ = S // 4         # 512 queries per core
MT = S // 128       # 16 key m-tiles
CN = 256            # n-chunk (queries per attention pass)
NP = NQ // CN       # 2 passes
SCALE = 1.0 / np.sqrt(np.float32(D))  # 1/32

_CACHE = {}


def _build():
    nc = bacc.Bacc()

    xt8a_d = nc.dram_tensor("xt8a", [128, 8, 512], FP8, kind="ExternalInput")
    xt8b_d = nc.dram_tensor("xt8b", [128, 8, 1536], FP8, kind="ExternalInput")
    xtba_d = nc.dram_tensor("xtba", [128, 8, 512], F16, kind="ExternalInput")
    xtbb_d = nc.dram_tensor("xtbb", [128, 8, 1536], F16, kind="ExternalInput")
    wq_d = nc.dram_tensor("wq", [128, 8, D], F16, kind="ExternalInput")
    wk8_d = nc.dram_tensor("wk8", [128, 8, D], FP8, kind="ExternalInput")
    wv_d = nc.dram_tensor("wv", [128, 8, D], F16, kind="ExternalInput")
    bqs_d = nc.dram_tensor("bqs", [128, G], F32, kind="ExternalInput")
    bks_d = nc.dram_tensor("bks", [128, G], F32, kind="ExternalInput")
    bvt_d = nc.dram_tensor("bvt", [1, D], F16, kind="ExternalInput")
    ident_d = nc.dram_tensor("ident", [128, 128], F16, kind="ExternalInput")
    out_d = nc.dram_tensor("ctxT", [D, NQ], F16, kind="ExternalOutput")

    ident = mybir.ActivationFunctionType.Identity
    expf = mybir.ActivationFunctionType.Exp

    with tile.TileContext(nc) as tc:
        with (
            tc.tile_pool(name="big", bufs=1) as big,
            tc.tile_pool(name="small", bufs=1) as small,
            tc.tile_pool(name="ework", bufs=2) as ework,
            tc.tile_pool(name="zwork", bufs=2) as zwork,
            tc.tile_pool(name="sc", bufs=2, space="PSUM") as scp,
            tc.tile_pool(name="cx", bufs=1, space="PSUM") as cxp,
        ):
            # ---- input DMAs, all contiguous on both sides (X pre-split
            # into query-prefix a / rest b): K prefix on sync queue first
            wk8_s = big.tile([128, 8, D], FP8)
            xt8a_s = big.tile([128, 8, 512], FP8)
            xt8b_s = big.tile([128, 8, 1536], FP8)
            wq_s = big.tile([128, 8, D], F16)
            xtba_s = big.tile([128, 8, 512], F16)
            xtbb_s = big.tile([128, 8, 1536], F16)
            wv_s = big.tile([128, 8, D], F16)

            nc.sync.dma_start(xt8a_s[:], xt8a_d[:])
            nc.sync.dma_start(wk8_s[:], wk8_d[:])
            nc.sync.dma_start(xt8b_s[:], xt8b_d[:])

            bks_s = small.tile([128, G], F32)
            nc.gpsimd.dma_start(bks_s[:], bks_d[:])
            bqs_s = small.tile([128, G], F32)
            nc.gpsimd.dma_start(bqs_s[:], bqs_d[:])
            bvt_s = small.tile([1, D], F16)
            nc.gpsimd.dma_start(bvt_s[:], bvt_d[:])
            ident_s = small.tile([128, 128], F16)
            nc.gpsimd.dma_start(ident_s[:], ident_d[:])
            nc.gpsimd.dma_start(wq_s[:], wq_d[:])
            nc.scalar.dma_start(xtba_s[:], xtba_d[:])
            nc.scalar.dma_start(wv_s[:], wv_d[:])
            nc.scalar.dma_start(xtbb_s[:], xtbb_d[:])

            def xt8_at(c0):
                # fp8 X^T columns start c0 as (tile, local col offset)
                return (xt8a_s, c0) if c0 < 512 else (xt8b_s, c0 - 512)

            def xtb_at(c0):
                return (xtba_s, c0) if c0 < 512 else (xtbb_s, c0 - 512)

            ones_s = small.tile([1, 128], F16)
            nc.vector.memset(ones_s[:], 1.0)

            kt_s = big.tile([128, G, S], F16)        # [h, g, m]
            v_s = big.tile([128, MT, D], F16)        # [m, mt, g*128+h] (32x)
            qt_s = big.tile([128, G, NQ], F16)       # [h, g, n]
            ctxt_s = big.tile([128, G, CN], F16)     # [h, g, n-chunk]
            partial0 = big.tile([128, G, CN], F16)   # p0 ctx over m-tiles 0-7
            partial1 = big.tile([128, G, CN], F16)   # p1 ctx over m-tiles 0-7
            bvb_s = small.tile([128, D], F16)        # 32*bv broadcast

            # ---- 32*bv broadcast tile via one-time ones-matmul ---------
            bvp = scp.tile([128, D], F32, tag="sc")
            nc.tensor.matmul(bvp[:, 0:512], ones_s[:], bvt_s[:, 0:512],
                             start=True, stop=True)
            nc.tensor.matmul(bvp[:, 512:1024], ones_s[:], bvt_s[:, 512:1024],
                             start=True, stop=True)
            nc.scalar.activation(bvb_s[:], bvp[:], ident)

            def k_chunk(mc):
                # K^T columns mc*512..+512, all groups; fp8 DoubleRow
                xs, c0 = xt8_at(mc * 512)
                for g in range(G):
                    kp = scp.tile([128, 512], F32, tag="sc")
                    for i in range(4):
                        nc.tensor.matmul(
                            kp[:],
                            wk8_s[:, 2 * i : 2 * i + 2, g * 128 : (g + 1) * 128],
                            xs[:, 2 * i : 2 * i + 2, c0 : c0 + 512],
                            start=(i == 0), stop=(i == 3), perf_mode=DR,
                        )
                    nc.scalar.activation(
                        kt_s[:, g, mc * 512 : (mc + 1) * 512], kp[:], ident,
                        bias=bks_s[:, g : g + 1], scale=1.0 / 32.0,
                    )

            def q_proj():
                for g in range(G):
                    qp = scp.tile([128, NQ], F32, tag="sc")
                    for dt in range(8):
                        nc.tensor.matmul(
                            qp[:],
                            wq_s[:, dt, g * 128 : (g + 1) * 128],
                            xtba_s[:, dt, 0:NQ],
                            start=(dt == 0), stop=(dt == 7),
                        )
                    nc.scalar.activation(
                        qt_s[:, g, :], qp[:], ident,
                        bias=bqs_s[:, g : g + 1], scale=float(SCALE),
                    )

            # V work units: one (mt, hc) chain + DVE bias-add per unit
            def v_unit(mt, hc):
                def emit():
                    xs, c0 = xtb_at(mt * 128)
                    vp = scp.tile([128, 512], F32, tag="sc")
                    for dt in range(8):
                        nc.tensor.matmul(
                            vp[:],
                            xs[:, dt, c0 : c0 + 128],
                            wv_s[:, dt, hc * 512 : (hc + 1) * 512],
                            start=(dt == 0), stop=(dt == 7),
                        )
                    nc.vector.tensor_add(
                        v_s[:, mt, hc * 512 : (hc + 1) * 512], vp[:],
                        bvb_s[:, hc * 512 : (hc + 1) * 512],
                    )
                return emit

            vq = [v_unit(mt, hc) for mt in range(MT) for hc in range(2)]
            vq_pos = [0]

            def pop_v(k=1):
                for _ in range(k):
                    if vq_pos[0] < len(vq):
                        vq[vq_pos[0]]()
                        vq_pos[0] += 1

            def ctx_mms(par_mt, e2, par, ctx_acc, gs, mlast):
                # ctx^T accumulation: out[h, n] += V_g^T @ P_g^T
                # Two groups share each 2KB PSUM bank. start=True resets the
                # whole bank's has_written bits, so only the first group of
                # each bank pair may issue it; the second group's first write
                # lands on cleared bits and overwrites, later writes accumulate.
                # In merge half-passes (m-tiles 8-15) the identity matmul
                # issued the start instead (par_mt >= 8 never matches 0).
                for g in gs:
                    nc.tensor.matmul(
                        ctx_acc[:, g, :],
                        v_s[:, par_mt, g * 128 : (g + 1) * 128],
                        e2[:, par, g, :],
                        start=(par_mt == 0 and g % 2 == 0),
                        stop=(par_mt == mlast),
                        skip_group_check=True,
                    )

            def chain(e2, pass1):
                # group-softmax normalization for a pair: Z-tree + recip on
                # DVE; 1/Z cast on Pool (pass0) / ACT (pass1); E*=1/Z on
                # DVE (all 8 groups pass0; 6 groups pass1, tail on Pool)
                t1p = zwork.tile([128, 2, 4, CN], F16, tag="t1")
                nc.vector.tensor_add(t1p[:], e2[:, :, 0:4, :], e2[:, :, 4:8, :])
                t2p = zwork.tile([128, 2, 2, CN], F16, tag="t2", bufs=1)
                nc.vector.tensor_add(t2p[:], t1p[:, :, 0:2, :], t1p[:, :, 2:4, :])
                zp = zwork.tile([128, 2, CN], F32, tag="z", bufs=1)
                nc.vector.tensor_add(zp[:], t2p[:, :, 0, :], t2p[:, :, 1, :])
                wp = zwork.tile([128, 2, CN], F32, tag="w", bufs=1)
                nc.vector.reciprocal_approx_fast(out=wp[:], in_=zp[:])
                wb = zwork.tile([128, 2, CN], F16, tag="wb", bufs=1)
                if pass1:
                    nc.scalar.activation(wb[:], wp[:], ident)
                else:
                    nc.gpsimd.tensor_copy(wb[:], wp[:])

                def wb_bcast(gc):
                    return bass.AP(
                        tensor=wb.tensor, offset=wb.offset,
                        ap=[wb.ap[0], wb.ap[1], [0, gc], wb.ap[2]],
                    )

                if pass1:
                    nc.vector.tensor_mul(
                        e2[:, :, 0:6, :], e2[:, :, 0:6, :], wb_bcast(6)
                    )
                    nc.gpsimd.tensor_mul(
                        e2[:, :, 6:8, :], e2[:, :, 6:8, :], wb_bcast(2)
                    )
                else:
                    nc.vector.tensor_mul(e2[:], e2[:], wb_bcast(8))
                return e2

            out_r = out_d.rearrange("(t p) n -> p t n", p=128)

            def emit_half(np_, mh, ctx_acc, kcs, vstep, drain_v,
                          merge_from, to_partial):
                """One half-pass: n-chunk np_, m-tiles mh*8..mh*8+7.

                kcs: K chunks to emit (at local pairs 0/2); vstep: pop a V
                unit every vstep fills; merge_from: SBUF partial to re-seed
                the accumulator via identity matmuls; to_partial: spill the
                half-sum to SBUF instead of emitting output.
                """
                n0 = np_ * CN
                mbase, mlast = mh * 8, mh * 8 + 7
                if merge_from is not None:
                    for g in range(G):
                        nc.tensor.matmul(
                            ctx_acc[:, g, :], ident_s[:], merge_from[:, g, :],
                            start=(g % 2 == 0), stop=False,
                            skip_group_check=True,
                        )
                pend = []
                nfill = [0]

                def fill(fi, _p):
                    nfill[0] += 1
                    if vstep and nfill[0] % vstep == 0:
                        pop_v(1)
                    if _p is not None:
                        par, gh = divmod(fi, 2)
                        ctx_mms(mbase + 2 * _p[0] + par, _p[1], par, ctx_acc,
                                range(gh * 4, gh * 4 + 4), mlast)

                for lp in range(4):
                    if kcs and lp % 2 == 0:
                        k_chunk(kcs.pop(0))
                    e2 = ework.tile([128, 2, G, CN], F16, tag="e2")
                    prev = pend.pop(0) if pend else None
                    for par in range(2):
                        mt = mbase + 2 * lp + par
                        for half in range(2):
                            sp = scp.tile([128, 4, CN], F32, tag="sc")
                            for gl in range(4):
                                g = half * 4 + gl
                                nc.tensor.matmul(
                                    sp[:, gl, :],
                                    kt_s[:, g, mt * 128 : (mt + 1) * 128],
                                    qt_s[:, g, n0 : n0 + CN],
                                    start=True, stop=True,
                                )
                            nc.scalar.activation(
                                e2[:, par, half * 4 : half * 4 + 4, :], sp[:], expf
                            )
                            fill(par * 2 + half, prev)
                    pend.append((lp, chain(e2, pass1=(np_ == 1 or mh == 1))))
                pop_v(drain_v)  # PE filler while the last chain completes
                for lp_, e2_ in pend:
                    for par in range(2):
                        ctx_mms(mbase + 2 * lp_ + par, e2_, par, ctx_acc,
                                range(G), mlast)
                if to_partial is not None:
                    nc.scalar.activation(
                        to_partial[:, 0:4, :], ctx_acc[:, 0:4, :], ident
                    )
                    nc.scalar.activation(
                        to_partial[:, 4:8, :], ctx_acc[:, 4:8, :], ident
                    )
                else:
                    # scale 1/32 undoes the 32x V scaling
                    nc.scalar.activation(
                        ctxt_s[:, 0:4, :], ctx_acc[:, 0:4, :], ident,
                        scale=1.0 / 32.0,
                    )
                    nc.sync.dma_start(
                        out_r[:, 0:4, n0 : n0 + CN], ctxt_s[:, 0:4, :]
                    )
                    nc.scalar.activation(
                        ctxt_s[:, 4:8, :], ctx_acc[:, 4:8, :], ident,
                        scale=1.0 / 32.0,
                    )
                    nc.sync.dma_start(
                        out_r[:, 4:8, n0 : n0 + CN], ctxt_s[:, 4:8, :]
                    )

            # Half-pass schedule: K/V projection work spreads over the first
            # three half-passes so only the last one is softmax-paced.
            k_chunk(0)
            q_proj()
            acc = cxp.tile([128, G, CN], F32, tag="cx")
            emit_half(0, 0, acc, kcs=[1, 2], vstep=1, drain_v=2,
                      merge_from=None, to_partial=partial0)
            acc = cxp.tile([128, G, CN], F32, tag="cx")
            emit_half(1, 0, acc, kcs=[3], vstep=2, drain_v=2,
                      merge_from=None, to_partial=partial1)
            acc = cxp.tile([128, G, CN], F32, tag="cx")
            emit_half(0, 1, acc, kcs=[], vstep=2, drain_v=4,
                      merge_from=partial0, to_partial=None)
            acc = cxp.tile([128, G, CN], F32, tag="cx")
            emit_half(1, 1, acc, kcs=[], vstep=0, drain_v=0,
                      merge_from=partial1, to_partial=None)

    nc.compile()
    return nc


def _ptx(a):
    # (t*128+p, cols) -> contiguous [p, t, cols] so SBUF DMAs are linear
    return np.ascontiguousarray(a.reshape(8, 128, -1).transpose(1, 0, 2))


def _prep_inputs(hidden_states, Wq, bq, Wk, bk, Wv, bv):
    f16 = np.float16
    f8 = ml_dtypes.float8_e4m3
    wq16 = _ptx(np.asarray(Wq, np.float32)).astype(f16)
    wk8 = _ptx(np.asarray(Wk, np.float32) * 32.0).astype(f8)
    wv16 = _ptx(np.asarray(Wv, np.float32) * 32.0).astype(f16)
    bqs = np.ascontiguousarray(
        (np.asarray(bq, np.float32) * SCALE).reshape(G, 128).T
    )
    bks = np.ascontiguousarray(np.asarray(bk, np.float32).reshape(G, 128).T)
    bvt = (np.asarray(bv, np.float32) * 32.0).astype(f16).reshape(1, D)

    identm = np.eye(128, dtype=f16)
    in_maps = []
    for core in range(8):
        b, j = divmod(core, 4)
        xt = np.asarray(hidden_states[b], np.float32).T  # (D, S)
        xt = _ptx(np.roll(xt, -j * NQ, axis=1))          # [p, t, m] queries 1st
        xa = np.ascontiguousarray(xt[:, :, 0:512])
        xb = np.ascontiguousarray(xt[:, :, 512:2048])
        in_maps.append(
            {
                "xt8a": xa.astype(f8), "xt8b": xb.astype(f8),
                "xtba": xa.astype(f16), "xtbb": xb.astype(f16),
                "wq": wq16, "wk8": wk8, "wv": wv16,
                "bqs": bqs, "bks": bks, "bvt": bvt, "ident": identm,
            }
        )
    return in_maps


def kernel(hidden_states, Wq, bq, Wk, bk, Wv, bv, _trace=False, _tmpdir=None):
    if "nc" not in _CACHE:
        _CACHE["nc"] = _build()
    nc = _CACHE["nc"]
    in_maps = _prep_inputs(hidden_states, Wq, bq, Wk, bk, Wv, bv)
    res = run_bass_kernel_spmd(
        nc, in_maps, list(range(8)), trace=_trace,
        **({"tmpdir": _tmpdir} if _tmpdir else {}),
    )
    _CACHE["last_result"] = res
    out = np.empty((B, S, D), np.float32)
    for core in range(8):
        b, j = divmod(core, 4)
        out[b, j * NQ : (j + 1) * NQ, :] = (
            res.results[core]["ctxT"].astype(np.float32).T
        )
    return out
